# revision 17
# baseline (speedup 1.0000x reference)
"""Decode-style single-query attention (B=32, N=8192, D=256, H=8) on 8 TRN2 cores.

Strategy: pure data-parallel over batch (4 batches/core, no collectives).
Per batch, the single query makes K/V projections unnecessary:
  scores[n,h] = X[n,:] @ kq[:,h],  kq = Wk-head-blocks @ (q@Wq + bq)  (bk cancels in softmax)
  pooled[h,:] = softmax(scores*scale)[h,:] @ X[n,:]   (flash-style, one pass over X)
  attn[e]    = pooled[e//32,:] @ Wv[:,e] + bv[e]
  out        = q_raw + attn @ Wo + bo
X streams through SBUF once as bf16 (cast during DMA). The scores matmul needs
X transposed (contraction over d): produced on-chip via the xbar DMA-transpose
(or PE transpose, switchable). The softmax denominator accumulates for free as
a ones-column appended to the pooling matmul's moving operand.
"""

import os
import sys

sys.path.insert(0, "/opt/trn_rl_repo")

from contextlib import ExitStack

import ml_dtypes
import numpy as np

import concourse.bass as bass
import concourse.tile as tile
from concourse import bacc, mybir
from concourse.bass_utils import run_bass_kernel_spmd

F32 = mybir.dt.float32
BF16 = mybir.dt.bfloat16
ts = bass.ts

B, D, H = 32, 256, 8
N = int(os.environ.get("K_N", "8192"))
DH = D // H
NCORES = 8
BL = B // NCORES  # batches per core
SCALE = 1.0 / float(np.sqrt(DH))

SLAB = int(os.environ.get("K_SLAB", "2048"))  # rows of X per streamed slab
NSUB = SLAB // 128  # 128-row subtiles per slab
NSLAB = N // SLAB  # slabs per batch
XT_MODE = os.environ.get("K_XT_MODE", "xbar")  # 'xbar' | 'pe'
CAST_MODE = os.environ.get("K_CAST", "dma")  # 'dma' (SWDGE cast) | 'dve'

EXP = mybir.ActivationFunctionType.Exp

_cache = {}


def build_graph(reps=1):
    nc = bacc.Bacc("TRN2", target_bir_lowering=False, debug=False, num_devices=NCORES)

    x_ext = nc.declare_dram_parameter("x", [BL, N, D], F32, isOutput=False)
    wq_ext = nc.declare_dram_parameter("Wq", [D, D], F32, isOutput=False)
    wkT_ext = nc.declare_dram_parameter("WkT", [D, D], F32, isOutput=False)
    wv_ext = nc.declare_dram_parameter("Wv", [D, D], F32, isOutput=False)
    wo_ext = nc.declare_dram_parameter("Wo", [D, D], F32, isOutput=False)
    bqc_ext = nc.declare_dram_parameter("bqc", [128, 2], F32, isOutput=False)
    bvc_ext = nc.declare_dram_parameter("bvc", [128, 2], F32, isOutput=False)
    bo_ext = nc.declare_dram_parameter("bo", [1, D], F32, isOutput=False)
    mqc_ext = nc.declare_dram_parameter("mqc", [128, 2, H], F32, isOutput=False)
    mh_ext = nc.declare_dram_parameter("maskh", [H, D], F32, isOutput=False)
    ones16_ext = nc.declare_dram_parameter("ones16", [128, 1], BF16, isOutput=False)
    id32_ext = nc.declare_dram_parameter("ident32", [128, 128], F32, isOutput=False)
    id16_ext = nc.declare_dram_parameter("ident16", [128, 128], BF16, isOutput=False)
    out_ext = nc.declare_dram_parameter("out", [BL, D], F32, isOutput=True)

    with tile.TileContext(nc) as tc, ExitStack() as ctx:
        const = ctx.enter_context(tc.tile_pool(name="const", bufs=1))
        stage = ctx.enter_context(tc.tile_pool(name="stage", bufs=1))
        xbp = ctx.enter_context(tc.tile_pool(name="xb", bufs=3))
        xtp = ctx.enter_context(tc.tile_pool(name="xt", bufs=3))
        pp = ctx.enter_context(tc.tile_pool(name="p", bufs=8))
        ep = ctx.enter_context(tc.tile_pool(name="ep", bufs=2))
        sps = ctx.enter_context(tc.tile_pool(name="sps", bufs=4, space="PSUM"))
        accp = ctx.enter_context(tc.tile_pool(name="accp", bufs=2, space="PSUM"))
        epsum = ctx.enter_context(tc.tile_pool(name="epsum", bufs=2, space="PSUM"))
        if XT_MODE == "pe":
            xtps = ctx.enter_context(tc.tile_pool(name="xtps", bufs=2, space="PSUM"))

        # ---- constants ----
        wq_sb = const.tile([128, 2, D], F32)  # Wq[d,e] d-chunked
        nc.sync.dma_start(wq_sb[:], wq_ext.ap().rearrange("(c p) e -> p c e", p=128))

        wkT_st = stage.tile([128, 2, D], F32, tag="stage")
        nc.sync.dma_start(wkT_st[:], wkT_ext.ap().rearrange("(c p) d -> p c d", p=128))
        wkT16 = const.tile([128, 2, D], BF16)  # WkT[e,d] e-chunked
        nc.vector.tensor_copy(wkT16[:], wkT_st[:])

        wv_st = stage.tile([128, 2, D], F32, tag="stage")
        nc.sync.dma_start(wv_st[:], wv_ext.ap().rearrange("(c p) e -> p c e", p=128))
        wv16 = const.tile([128, 2, D], BF16)  # Wv[d,e] d-chunked
        nc.vector.tensor_copy(wv16[:], wv_st[:])

        wo_st = stage.tile([128, 2, D], F32, tag="stage")
        nc.sync.dma_start(wo_st[:], wo_ext.ap().rearrange("(c p) e -> p c e", p=128))
        wo16 = const.tile([128, 2, D], BF16)  # Wo[e,e'] e-chunked
        nc.vector.tensor_copy(wo16[:], wo_st[:])

        bqc_sb = const.tile([128, 2], F32)
        nc.sync.dma_start(bqc_sb[:], bqc_ext.ap())
        bvc_sb = const.tile([128, 2], F32)
        nc.sync.dma_start(bvc_sb[:], bvc_ext.ap())
        bo_sb = const.tile([1, D], F32)
        nc.sync.dma_start(bo_sb[:], bo_ext.ap())
        mqc_sb = const.tile([128, 2, H], F32)
        nc.sync.dma_start(mqc_sb[:], mqc_ext.ap())
        mh_sb = const.tile([H, D], F32)
        nc.sync.dma_start(mh_sb[:], mh_ext.ap())
        ones16_sb = const.tile([128, 1], BF16)
        nc.sync.dma_start(ones16_sb[:], ones16_ext.ap())
        id32_sb = const.tile([128, 128], F32)
        nc.sync.dma_start(id32_sb[:], id32_ext.ap())
        id16_sb = const.tile([128, 128], BF16)
        nc.sync.dma_start(id16_sb[:], id16_ext.ap())

        for b in [bb for _ in range(reps) for bb in range(BL)]:
            # ---- per-batch prologue: q, kq ----
            qT = ep.tile([128, 2], F32, tag="qT")  # x[b,0,:] as column chunks
            nc.sync.dma_start(qT[:], x_ext.ap()[b, 0, :].rearrange("(c p) -> p c", p=128))
            qn = ep.tile([1, D], F32, tag="qn")  # x[b,0,:] natural
            nc.sync.dma_start(qn[:], x_ext.ap()[b, 0:1, :])
            qbo = ep.tile([1, D], F32, tag="qbo")
            nc.vector.tensor_add(qbo[:], qn[:], bo_sb[:])

            qf_ps = epsum.tile([128, 2], F32, tag="eps")
            for mc in range(2):
                for kc in range(2):
                    nc.tensor.matmul(
                        qf_ps[:, mc : mc + 1],
                        wq_sb[:, kc, ts(mc, 128)],
                        qT[:, kc : kc + 1],
                        start=(kc == 0),
                        stop=(kc == 1),
                    )
            qfb = ep.tile([128, 2], F32, tag="qfb")
            nc.vector.tensor_add(qfb[:], qf_ps[:], bqc_sb[:])

            sq16 = ep.tile([128, 2, H], BF16, tag="sq16")
            for c in range(2):
                nc.vector.tensor_scalar_mul(sq16[:, c, :], mqc_sb[:, c, :], qfb[:, c : c + 1])

            kqT_ps = epsum.tile([H, D], F32, tag="eps")
            for c in range(2):
                nc.tensor.matmul(
                    kqT_ps[:], sq16[:, c, :], wkT16[:, c, :], start=(c == 0), stop=(c == 1)
                )
            kqT_sb = ep.tile([H, D], F32, tag="kqT")
            nc.vector.tensor_copy(kqT_sb[:], kqT_ps[:])

            kq_ps = epsum.tile([128, 2, H], F32, tag="eps")
            for c in range(2):
                nc.tensor.transpose(kq_ps[:, c, :], kqT_sb[:, ts(c, 128)], id32_sb[:H, :H])
            kq16 = ep.tile([128, 2, H], BF16, tag="kq16")
            for c in range(2):
                nc.vector.tensor_copy(kq16[:, c, :], kq_ps[:, c, :])

            # ---- stream X: scores -> exp -> pooled accumulation ----
            # Software-pipelined: scores/exp run LOOK subtiles ahead of the
            # pooled matmuls so PE never stalls waiting on ACT's exp.
            # acc_ps is one bank: cols 0:D pooled sums, col D softmax denom.
            # Both chains share one accumulation group (disjoint columns);
            # the group opens on the first pooled MM and closes on the last
            # denominator MM.
            acc_ps = accp.tile([H, D + 1], F32, tag="acc")
            LOOK = 4
            pend = []

            def emit_pooled(flush):
                p16, xbt, t, first = pend.pop(0)
                last = flush and not pend
                nc.tensor.matmul(
                    acc_ps[:, 0:D], p16[:], xbt[:, t, :], start=first, stop=False
                )
                nc.tensor.matmul(
                    acc_ps[:, D : D + 1],
                    p16[:],
                    ones16_sb[:, 0:1],
                    start=False,
                    stop=last,
                )

            for s in range(NSLAB):
                xb = xbp.tile([128, NSUB, D], BF16, tag="xb")
                src = x_ext.ap()[b, s * SLAB : (s + 1) * SLAB, :].rearrange(
                    "(t p) d -> p t d", p=128
                )
                if CAST_MODE == "dma":
                    nc.gpsimd.dma_start(xb[:], src)  # f32 -> bf16 cast in DMA
                else:
                    xf = xbp.tile([128, NSUB, D], F32, tag="xf")
                    nc.sync.dma_start(xf[:], src)
                    nc.vector.tensor_copy(xb[:], xf[:])

                # xt[:, t*2+c, :] = X[t*128: (t+1)*128, c*128:(c+1)*128].T
                xt = xtp.tile([128, 2 * NSUB, 128], BF16, tag="xt")
                if XT_MODE == "xbar":
                    nc.sync.dma_start(
                        out=xt[:], in_=xb[:].rearrange("p t d -> p (t d)"),
                        transpose=True,
                    )
                else:
                    for half in range(NSUB // 2):
                        tp = xtps.tile([128, 512], BF16, tag="xtps")
                        for j in range(4):
                            t = half * 2 + j // 2
                            c = j % 2
                            nc.tensor.transpose(
                                tp[:, ts(j, 128)], xb[:, t, ts(c, 128)], id16_sb[:]
                            )
                        if half % 2 == 0:
                            nc.vector.tensor_copy(xt[:, ts(half, 4), :], tp[:])
                        else:
                            nc.scalar.copy(xt[:, ts(half, 4), :], tp[:])

                for t in range(NSUB):
                    s_ps = sps.tile([128, H], F32, tag="s")
                    for c in range(2):
                        nc.tensor.matmul(
                            s_ps[:],
                            xt[:, t * 2 + c, :],
                            kq16[:, c, :],
                            start=(c == 0),
                            stop=(c == 1),
                        )
                    p16 = pp.tile([128, H], BF16, tag="p")
                    nc.scalar.activation(p16[:], s_ps[:], EXP, scale=SCALE)
                    pend.append((p16, xb, t, s == 0 and t == 0))
                    if len(pend) > LOOK:
                        emit_pooled(False)
            while pend:
                emit_pooled(True)

            # ---- per-batch epilogue ----
            linv = ep.tile([H, 1], F32, tag="linv")
            nc.vector.reciprocal(linv[:], acc_ps[:, D : D + 1])
            pooled16 = ep.tile([H, D], BF16, tag="pooled")
            nc.vector.tensor_scalar_mul(pooled16[:], acc_ps[:, 0:D], linv[:, 0:1])

            pt_ps = epsum.tile([128, 2, H], BF16, tag="eps")
            for c in range(2):
                nc.tensor.transpose(pt_ps[:, c, :], pooled16[:, ts(c, 128)], id16_sb[:H, :H])
            pt16 = ep.tile([128, 2, H], BF16, tag="pt16")
            for c in range(2):
                nc.vector.tensor_copy(pt16[:, c, :], pt_ps[:, c, :])

            y_ps = epsum.tile([H, D], F32, tag="eps")
            for c in range(2):
                nc.tensor.matmul(
                    y_ps[:], pt16[:, c, :], wv16[:, c, :], start=(c == 0), stop=(c == 1)
                )
            ym16 = ep.tile([H, D], BF16, tag="ym")
            nc.vector.tensor_mul(ym16[:], y_ps[:], mh_sb[:])

            attn_ps = epsum.tile([1, D], F32, tag="eps")
            nc.tensor.matmul(attn_ps[:], ones16_sb[:H, 0:1], ym16[:], start=True, stop=True)
            attn_sb = ep.tile([1, D], F32, tag="attn")
            nc.vector.tensor_copy(attn_sb[:], attn_ps[:])

            at_ps = epsum.tile([128, 2], F32, tag="eps")
            for c in range(2):
                nc.tensor.transpose(at_ps[:, c : c + 1], attn_sb[:, ts(c, 128)], id32_sb[:1, :1])
            at16 = ep.tile([128, 2], BF16, tag="at16")
            for c in range(2):
                nc.vector.tensor_add(at16[:, c : c + 1], at_ps[:, c : c + 1], bvc_sb[:, c : c + 1])

            res_ps = epsum.tile([1, D], F32, tag="eps")
            for c in range(2):
                nc.tensor.matmul(
                    res_ps[:], at16[:, c : c + 1], wo16[:, c, :], start=(c == 0), stop=(c == 1)
                )
            out_sb = ep.tile([1, D], F32, tag="out")
            nc.vector.tensor_add(out_sb[:], res_ps[:], qbo[:])
            nc.sync.dma_start(out_ext.ap()[b : b + 1, :], out_sb[:])

    nc.compile()
    return nc


def _host_consts():
    e = np.arange(D)
    mq = (e[:, None] // DH == np.arange(H)[None, :]).astype(np.float32)  # [D, H]
    consts = {
        "mqc": np.ascontiguousarray(mq.reshape(2, 128, H).transpose(1, 0, 2)),
        "maskh": np.ascontiguousarray((np.arange(H)[:, None] == e[None, :] // DH).astype(np.float32)),
        "ones16": np.ones((128, 1), ml_dtypes.bfloat16),
        "ident32": np.eye(128, dtype=np.float32),
        "ident16": np.eye(128, dtype=ml_dtypes.bfloat16),
    }
    return consts


def kernel(**inputs):
    x = np.ascontiguousarray(np.asarray(inputs["x"], dtype=np.float32))
    Wq = np.ascontiguousarray(np.asarray(inputs["Wq"], dtype=np.float32))
    bq = np.asarray(inputs["bq"], dtype=np.float32)
    Wk = np.ascontiguousarray(np.asarray(inputs["Wk"], dtype=np.float32))
    Wv = np.ascontiguousarray(np.asarray(inputs["Wv"], dtype=np.float32))
    Wo = np.ascontiguousarray(np.asarray(inputs["Wo"], dtype=np.float32))
    bv = np.asarray(inputs["bv"], dtype=np.float32)
    bo = np.asarray(inputs["bo"], dtype=np.float32)
    # bk is unused: softmax is shift-invariant and Q.bk is constant over keys.

    if "nc" not in _cache:
        _cache["nc"] = build_graph()
    nc = _cache["nc"]

    consts = _host_consts()
    shared = {
        "Wq": Wq,
        "WkT": np.ascontiguousarray(Wk.T),
        "Wv": Wv,
        "Wo": Wo,
        "bqc": np.ascontiguousarray(bq.reshape(2, 128).T),
        "bvc": np.ascontiguousarray(bv.reshape(2, 128).T),
        "bo": np.ascontiguousarray(bo.reshape(1, D)),
        **consts,
    }
    in_maps = []
    for c in range(NCORES):
        m = dict(shared)
        m["x"] = np.ascontiguousarray(x[c * BL : (c + 1) * BL])
        in_maps.append(m)

    trace = bool(int(os.environ.get("K_TRACE", "0")))
    if trace:
        try:
            import axon_prof

            axon_prof.install()
        except Exception as e:
            print(f"axon_prof install failed: {e}")
    res = run_bass_kernel_spmd(
        nc,
        in_maps,
        core_ids=list(range(NCORES)),
        trace=trace,
        tmpdir=os.environ.get("K_TRACE_DIR") or None,
    )
    _cache["last_results"] = res
    out = np.concatenate([res.results[i]["out"] for i in range(NCORES)], axis=0)
    return out.reshape(B, 1, D).astype(np.float32)
